# revision 1
# baseline (speedup 1.0000x reference)
"""BinMNIST forward — 8-core data-parallel kernel.

Sharding: pure data parallel per the hint. Batch 2048 -> 8 shards of 256.
Weights replicated. shift_bn batch statistics are computed with cross-core
all-reduces (lax.pmean over the device axis), so results match the
full-batch reference semantics. Self-contained: no sibling imports.
"""
import os

os.environ.setdefault("NEURON_CC_FLAGS", "--auto-cast=none")

import numpy as np
import jax
import jax.numpy as jnp
from functools import partial

EPS = 1e-4
N_CORES = 8
B = 2048


def _binarize(x):
    return jnp.where(x >= 0, 1.0, -1.0).astype(x.dtype)


def _ap2(x):
    a = jnp.maximum(jnp.abs(x), 1e-38)
    return jnp.sign(x) * jnp.exp2(jnp.round(jnp.log2(a)))


def _shift_bn_sharded(x, gamma, beta, axes, axis_name):
    # mean over (local axes) then over cores -> exact global batch mean
    mean = jnp.mean(x, axis=axes, keepdims=True)
    if axis_name is not None:
        mean = jax.lax.pmean(mean, axis_name)
    c = x - mean
    var = jnp.mean(c * _ap2(c), axis=axes, keepdims=True)
    if axis_name is not None:
        var = jax.lax.pmean(var, axis_name)
    xhat = c * _ap2(1.0 / jnp.sqrt(var + EPS))
    shape = [1] * x.ndim
    shape[1] = x.shape[1]
    return _ap2(gamma.reshape(shape)) * xhat + beta.reshape(shape)


def _conv_bin(x, w, b):
    y = jax.lax.conv_general_dilated(
        x, _binarize(w), (1, 1), [(1, 1), (1, 1)],
        dimension_numbers=("NCHW", "OIHW", "NCHW"))
    return y + b[None, :, None, None]


def _maxpool3s2(x):
    return jax.lax.reduce_window(
        x, -jnp.inf, jax.lax.max, (1, 1, 3, 3), (1, 1, 2, 2), "VALID")


def _forward(x, conv1_w, conv1_b, g1, b1, conv2_w, conv2_b, g2, b2,
             lin3_w, lin3_b, g3, b3, lin4_w, lin4_b, axis_name=None):
    h = x.reshape(-1, 1, 28, 28)
    h = jax.nn.relu(_conv_bin(h, conv1_w, conv1_b))
    h = _shift_bn_sharded(h, g1, b1, (0, 2, 3), axis_name)
    h = _binarize(h)
    h = jax.nn.relu(_conv_bin(h, conv2_w, conv2_b))
    h = _maxpool3s2(h)
    h = _shift_bn_sharded(h, g2, b2, (0, 2, 3), axis_name)
    h = _binarize(h)
    h = h.reshape(h.shape[0], -1)
    h = jax.nn.relu(h @ _binarize(lin3_w).T + lin3_b)
    h = _shift_bn_sharded(h, g3, b3, (0,), axis_name)
    h = _binarize(h)
    return h @ _binarize(lin4_w).T + lin4_b


_ORDER = ("conv1_w", "conv1_b", "g1", "b1", "conv2_w", "conv2_b", "g2", "b2",
          "lin3_w", "lin3_b", "g3", "b3", "lin4_w", "lin4_b")


def kernel(**inputs):
    x = np.asarray(inputs["x"], dtype=np.float32)
    ws = [np.asarray(inputs[k], dtype=np.float32) for k in _ORDER]
    try:
        devs = [d for d in jax.devices() if d.platform != "cpu"][:N_CORES]
        if len(devs) < N_CORES:
            devs = jax.devices()[:N_CORES]
        assert len(devs) == N_CORES, f"need {N_CORES} devices, have {len(devs)}"
        xs = x.reshape(N_CORES, B // N_CORES, 784)
        fn = jax.pmap(
            partial(_forward, axis_name="i"),
            axis_name="i",
            in_axes=(0,) + (None,) * len(_ORDER),
            devices=devs,
        )
        out = fn(xs, *ws)
        out = np.asarray(out, dtype=np.float32).reshape(B, 10)
    except Exception:
        # Fallback: single-device execution with identical (full-batch) math.
        out = np.asarray(
            jax.jit(_forward)(jnp.asarray(x), *[jnp.asarray(w) for w in ws]),
            dtype=np.float32,
        )
    return out



# revision 14
# speedup vs baseline: 3.9166x; 3.9166x over previous
"""BinMNIST forward — 8-core Bass/Tile kernel for TRN2.

Sharding: conv stage data-parallel over batch (256 img/core); lin3 sharded
over units (256/core) after an fp8 AllGather of binarized conv features;
lin4 K-sharded with host-side final sum of the 8 partial [2048,10] outputs.

Numerics: binarized tensors are encoded as +-0.5 in fp8 (exact); matmul
outputs are exact dyadic rationals in f32 PSUM, so every binarize
threshold compare (>= mean) reproduces the reference's f32 sign decisions.
conv1 runs bf16 with a 3-way hi/mid/lo split of x (f32-exact products).
shift_bn+binarize reduces to a per-channel mean-threshold compare when
beta==0 and gamma>0 (checked on host; true for the graded inputs).
"""
import os
import sys
import numpy as np

sys.path.insert(0, "/opt/trn_rl_repo")

N_CORES = 8
B = 2048
BC = B // N_CORES          # 256 images per core
U = 2048

_CACHE = {}


def _build_nc():
    import concourse.bass as bass
    import concourse.mybir as mybir
    from concourse.tile import TileContext

    f32 = mybir.dt.float32
    bf16 = mybir.dt.bfloat16
    fp8 = mybir.dt.float8e4
    f16 = mybir.dt.float16
    ALU = mybir.AluOpType
    ACT = mybir.ActivationFunctionType
    AX = mybir.AxisListType

    nc = bass.Bass(num_devices=N_CORES)
    core_ids = list(range(N_CORES))

    # ---------------- DRAM I/O ----------------
    xpad = nc.dram_tensor("xpad", [BC, 3, 30, 30], bf16, kind="ExternalInput")
    w1l = nc.dram_tensor("w1l", [3, 7, 90, 128], bf16, kind="ExternalInput")
    b1v = nc.dram_tensor("b1v", [128, 1], f32, kind="ExternalInput")
    w2l = nc.dram_tensor("w2l", [9, 128, 128], fp8, kind="ExternalInput")
    w3i = nc.dram_tensor("w3i", [128, 85, 2, 128], fp8, kind="ExternalInput")
    b3v = nc.dram_tensor("b3v", [128, 2], f32, kind="ExternalInput")
    w4i = nc.dram_tensor("w4i", [2, 128, 10], fp8, kind="ExternalInput")
    out_p = nc.dram_tensor("out_p", [B, 10], f32, kind="ExternalOutput")

    # internal DRAM
    z1d = nc.dram_tensor("z1d", [7, 128, 7168], f32)            # conv1 relu spill
    s1_in = nc.dram_tensor("s1_in", [128], f32)
    s1_out = nc.dram_tensor("s1_out", [128], f32, addr_space="Shared")
    s2_in = nc.dram_tensor("s2_in", [128], f32)
    s2_out = nc.dram_tensor("s2_out", [128], f32, addr_space="Shared")
    h2c = nc.dram_tensor("h2c", [85, 2, 64, 256], fp8)          # local contribution
    h2g = nc.dram_tensor("h2g", [8, 85, 2, 64, 256], fp8, addr_space="Shared")

    INV_N1 = 1.0 / (B * 28 * 28)
    INV_N2 = 1.0 / (B * 13 * 13)
    INV_N3 = 1.0 / B

    with TileContext(nc) as tc:
        with (
            tc.tile_pool(name="const", bufs=1) as cpool,
            tc.tile_pool(name="work", bufs=3) as wpool,
            tc.tile_pool(name="work2", bufs=2) as wpool2,
            tc.tile_pool(name="big", bufs=1) as bigpool,
            tc.tile_pool(name="slabs", bufs=2) as slabpool,
            tc.tile_pool(name="rhs3p", bufs=3) as rpool,
        ):
            # ---- small resident tensors ----
            w1t = cpool.tile([90, 3, 7, 128], bf16, tag="w1t")
            nc.sync.dma_start(out=w1t[:, :, :, :], in_=w1l.rearrange("d m k n -> k d m n"))
            b1t = cpool.tile([128, 1], f32, tag="b1t")
            nc.sync.dma_start(out=b1t[:, :], in_=b1v[:, :])
            w2t = cpool.tile([128, 9, 128], fp8, tag="w2t")
            nc.sync.dma_start(out=w2t[:, :, :], in_=w2l.rearrange("t k n -> k t n"))
            b3t = cpool.tile([128, 2], f32, tag="b3t")
            nc.sync.dma_start(out=b3t[:, :], in_=b3v[:, :])
            w4t = cpool.tile([128, 2, 10], fp8, tag="w4t")
            nc.sync.dma_start(out=w4t[:, :, :], in_=w4i.rearrange("m k n -> k m n"))

            # xp shares the big "poolslot" slot (later reused by pooled, then w3t)
            xp = bigpool.tile([90, BC, 30], bf16, tag="poolslot")
            nc.sync.dma_start(out=xp[:, :, :], in_=xpad.rearrange("b s x y -> (s x) b y"))

            # =========== conv1: 7 M-tiles x 16 b-chunks x 3 dy ===========
            acc1 = cpool.tile([128, 7, 16], f32, tag="acc1")
            with tc.tile_pool(name="ps1", bufs=8, space="PSUM") as pp1:
                for mt in range(7):
                    for q in range(4):
                        stage = wpool.tile([128, 4, 16, 28], f32, tag="scr")
                        for s in range(4):
                            ch = q * 4 + s
                            ps = pp1.tile([128, 16, 28], f32, tag="ps1")
                            for dy in range(3):
                                nc.tensor.matmul(
                                    ps[:, :, :],
                                    w1t[:, dy, mt, :],
                                    xp[:, ch * 16:(ch + 1) * 16, dy:dy + 28],
                                    start=(dy == 0), stop=(dy == 2),
                                )
                            nc.scalar.activation(
                                stage[:, s, :, :], ps[:, :, :], ACT.Relu,
                                bias=b1t[:, 0:1], scale=1.0,
                                accum_out=acc1[:, mt, ch:ch + 1],
                            )
                        nc.sync.dma_start(
                            out=z1d[mt, :, q * 1792:(q + 1) * 1792],
                            in_=stage[:, :, :, :])

            # ---- bn1 stats: reduce + allreduce + fold to per-lane threshold ----
            s1t = cpool.tile([128, 1], f32, tag="s1t")
            nc.vector.tensor_reduce(s1t[:, :], acc1[:, :, :], AX.XY, ALU.add)
            nc.sync.dma_start(out=s1_in[:], in_=s1t[:, 0])
            nc.gpsimd.collective_compute(
                "AllReduce", ALU.add, replica_groups=[core_ids],
                ins=[s1_in[:]], outs=[s1_out[:]],
            )
            s1g = cpool.tile([128, 1], f32, tag="s1g")
            nc.sync.dma_start(out=s1g[:, :], in_=s1_out.rearrange("(p one) -> p one", one=1))
            t1 = cpool.tile([128, 1], f32, tag="t1")
            nc.vector.tensor_tensor(t1[0:32, :], s1g[0:32, :], s1g[32:64, :], ALU.add)
            nc.vector.tensor_tensor(t1[32:64, :], s1g[64:96, :], s1g[96:128, :], ALU.add)
            nc.vector.tensor_tensor(t1[0:32, :], t1[0:32, :], t1[32:64, :], ALU.add)
            nc.vector.tensor_scalar(t1[0:32, :], t1[0:32, :], INV_N1, None, ALU.mult)
            nc.vector.tensor_copy(t1[32:64, :], t1[0:32, :])
            nc.vector.tensor_copy(t1[64:96, :], t1[0:32, :])
            nc.vector.tensor_copy(t1[96:128, :], t1[0:32, :])

            # =========== bn1 compare + restripe to conv2 layout ===========
            # h1b4: [128=(4 bq,32 ic), (64 b, 30 xpad, 30 ypad)] fp8, borders 0
            h1b4 = bigpool.tile([128, 64, 30, 30], fp8, tag="bigA")
            nc.gpsimd.memset(h1b4[:, :, 0, :], 0.0)
            nc.gpsimd.memset(h1b4[:, :, 29, :], 0.0)
            nc.gpsimd.memset(h1b4[:, :, :, 0], 0.0)
            nc.gpsimd.memset(h1b4[:, :, :, 29], 0.0)
            for mt in range(7):
                for q in range(4):
                    zr = wpool.tile([128, 64, 28], f32, tag="scr")
                    nc.sync.dma_start(out=zr[:, :, :],
                                      in_=z1d[mt, :, q * 1792:(q + 1) * 1792])
                    bt = wpool.tile([128, 64, 28], fp8, tag="scr")
                    nc.gpsimd.tensor_scalar(bt[:, :, :], zr[:, :, :], t1[:, 0:1], 0.5,
                                            ALU.is_ge, ALU.subtract)
                    for xl in range(4):
                        nc.sync.dma_start(
                            out=h1b4[q * 32:(q + 1) * 32, :, 4 * mt + xl + 1, 1:29],
                            in_=bt[xl * 32:(xl + 1) * 32, :, :],
                        )

            # =========== conv2 (16-way tile_position, 9 taps) + pooling ===========
            pooled = bigpool.tile([128, 4, 13, 13, 32], f16, tag="poolslot")
            prev_slab = None
            with tc.tile_pool(name="ps2", bufs=2, space="PSUM") as pp2:
                for xg in range(7):
                    slab = slabpool.tile([128, 4, 4, 13, 32], f16, tag="myslab")
                    for bg in range(8):
                        pss = [pp2.tile([128, 4, 4, 28], f32, tag=f"ps2_{i}",
                                        name=f"ps2_{i}")
                               for i in range(4)]
                        for i in range(4):
                            for j in range(4):
                                g = j // 2
                                for tap in range(9):
                                    dx, dyy = tap // 3, tap % 3
                                    nc.tensor.matmul(
                                        pss[i][32 * j:32 * (j + 1), :, :, :],
                                        w2t[32 * i:32 * (i + 1), tap, 32 * j:32 * (j + 1)],
                                        h1b4[32 * i:32 * (i + 1),
                                             g * 32 + bg * 4: g * 32 + bg * 4 + 4,
                                             4 * xg + dx: 4 * xg + dx + 4,
                                             dyy: dyy + 28],
                                        start=(tap == 0), stop=(tap == 8),
                                        tile_position=(32 * i, 32 * j),
                                    )
                        # evict + y-pool (13 windows of 3, stride 2) — DVE only
                        for i in range(4):
                            ps = pss[i]
                            tmp = wpool2.tile([128, 4, 4, 13], f32, tag="pooltmp")
                            nc.vector.tensor_tensor(tmp[:, :, :, :], ps[:, :, :, 0:25:2],
                                                    ps[:, :, :, 1:26:2], ALU.max)
                            nc.vector.tensor_tensor(
                                slab[:, i, :, :, bg * 4:bg * 4 + 4]
                                    .rearrange("p x y b -> p b x y"),
                                tmp[:, :, :, :], ps[:, :, :, 2:27:2], ALU.max)
                    # rolling x-pool: windows w with (w+1)//2 == xg
                    for w in (2 * xg - 1, 2 * xg):
                        if w < 0 or w > 12:
                            continue
                        x0 = 2 * w
                        cols = []
                        for xx in (x0, x0 + 1, x0 + 2):
                            if xx >= 4 * xg:
                                cols.append(slab[:, :, xx - 4 * xg, :, :])
                            else:
                                cols.append(prev_slab[:, :, xx - 4 * (xg - 1), :, :])
                        pt = wpool2.tile([128, 4, 13, 32], f16, tag="xpt")
                        nc.vector.tensor_tensor(pt[:, :, :, :], cols[0], cols[1], ALU.max)
                        nc.vector.tensor_tensor(pooled[:, :, w, :, :], pt[:, :, :, :],
                                                cols[2], ALU.max)
                    prev_slab = slab

            # ---- bn2 stats ----
            s2t = cpool.tile([128, 1], f32, tag="s2t")
            nc.vector.tensor_reduce(s2t[:, :], pooled[:, :, :, :, :], AX.XYZW, ALU.add)
            nc.sync.dma_start(out=s2_in[:], in_=s2t[:, 0])
            nc.gpsimd.collective_compute(
                "AllReduce", ALU.add, replica_groups=[core_ids],
                ins=[s2_in[:]], outs=[s2_out[:]],
            )
            s2g = cpool.tile([128, 1], f32, tag="s2g")
            nc.sync.dma_start(out=s2g[:, :], in_=s2_out.rearrange("(p one) -> p one", one=1))
            t2 = cpool.tile([128, 1], f32, tag="t2")
            nc.vector.tensor_tensor(t2[0:64, :], s2g[0:64, :], s2g[64:128, :], ALU.add)
            nc.vector.tensor_scalar(t2[0:64, :], t2[0:64, :], INV_N2, None, ALU.mult)
            nc.vector.tensor_copy(t2[64:128, :], t2[0:64, :])

            # ---- bn2 compare (per 64-partition block) -> restripe -> AllGather ----
            zpad = cpool.tile([64, 256], fp8, tag="zpad")
            nc.vector.memset(zpad[:, :], 0.0)
            nc.sync.dma_start(out=h2c[84, 1, :, :], in_=zpad[:, :])
            for g in range(2):
                for i in range(4):
                    blk = wpool2.tile([64, 169, 32], fp8, tag="h2blk")
                    nc.gpsimd.tensor_scalar(
                        blk[:, :, :],
                        pooled.rearrange("p i xw yw b -> p i (xw yw) b")[
                            g * 64:(g + 1) * 64, i, :, :],
                        t2[g * 64:(g + 1) * 64, 0:1], 0.5, ALU.is_ge, ALU.subtract)
                    # even pixels -> par 0 (85 pairs), odd pixels -> par 1 (84 pairs)
                    dst = h2c.rearrange("j r c b -> c j r b")
                    nc.sync.dma_start(
                        out=dst[:, :, 0, i * 64 + g * 32: i * 64 + g * 32 + 32],
                        in_=blk[:, 0:169:2, :])
                    nc.sync.dma_start(
                        out=dst[:, 0:84, 1, i * 64 + g * 32: i * 64 + g * 32 + 32],
                        in_=blk[:, 1:168:2, :])
            nc.gpsimd.collective_compute(
                "AllGather", ALU.bypass, replica_groups=[core_ids],
                ins=[h2c[:, :, :, :]], outs=[h2g[:, :, :, :, :]],
            )

            # =========== lin3: 85 pixel-pairs, units sharded ===========
            w3t = bigpool.tile([128, 85, 2, 128], fp8, tag="poolslot")
            nc.sync.dma_start(out=w3t[:, :, :, :], in_=w3i[:, :, :, :])
            h3 = bigpool.tile([128, 2, B], f32, tag="bigA")
            with tc.tile_pool(name="ps3", bufs=1, space="PSUM") as pp3:
                ps3 = [pp3.tile([128, 4, 512], f32, tag=f"ps3_{mt}",
                                name=f"ps3_{mt}") for mt in range(2)]
                for j in range(85):
                    rhs = rpool.tile([128, 8, 256], fp8, tag="rhs3")
                    nc.sync.dma_start(
                        out=rhs[:, :, :],
                        in_=h2g.rearrange("s j r c b -> (r c) j s b")[:, j, :, :])
                    for mt in range(2):
                        for nch in range(4):
                            nc.tensor.matmul(
                                ps3[mt][:, nch, :],
                                w3t[:, j, mt, :],
                                rhs.rearrange("p (n y) b -> p n (y b)", n=4)[:, nch, :],
                                start=(j == 0), stop=(j == 84),
                            )
                for mt in range(2):
                    nc.scalar.activation(
                        h3.rearrange("p m (n x) -> p m n x", n=4)[:, mt, :, :],
                        ps3[mt][:, :, :], ACT.Relu,
                        bias=b3t[:, mt:mt + 1], scale=1.0)

            # ---- bn3 (local, no collective) ----
            s3t = cpool.tile([128, 2], f32, tag="s3t")
            nc.vector.tensor_reduce(s3t[:, :], h3[:, :, :], AX.X, ALU.add)
            t3 = cpool.tile([128, 2], f32, tag="t3")
            nc.vector.tensor_scalar(t3[:, :], s3t[:, :], INV_N3, None, ALU.mult)
            h3b = bigpool.tile([128, 2, B], fp8, tag="h3b")
            for mt in range(2):
                nc.vector.tensor_scalar(h3b[:, mt, :], h3[:, mt, :], t3[:, mt:mt + 1],
                                        0.5, ALU.is_ge, ALU.subtract)

            # =========== lin4 (flipped): out[b,10] partials ===========
            with tc.tile_pool(name="ps4", bufs=1, space="PSUM") as pp4:
                ps4 = pp4.tile([128, 16, 10], f32, tag="ps4")
                for bt4 in range(16):
                    for mt in range(2):
                        nc.tensor.matmul(
                            ps4[:, bt4, :],
                            h3b[:, mt, bt4 * 128:(bt4 + 1) * 128],
                            w4t[:, mt, :],
                            start=(mt == 0), stop=(mt == 1),
                        )
                ot = wpool2.tile([128, 16, 10], f32, tag="ot")
                nc.scalar.copy(ot[:, :, :], ps4[:, :, :])
                nc.sync.dma_start(
                    out=out_p.rearrange("(t p) n -> p t n", p=128), in_=ot[:, :, :])

    return nc


def _host_prep(inputs):
    import ml_dtypes
    bf16 = ml_dtypes.bfloat16
    fp8 = ml_dtypes.float8_e4m3

    x = np.asarray(inputs["x"], np.float32)
    sgn = lambda a: np.where(np.asarray(a, np.float32) >= 0,
                             np.float32(1.0), np.float32(-1.0))

    g_ok = (np.all(np.asarray(inputs["b1"]) == 0) and np.all(np.asarray(inputs["b2"]) == 0)
            and np.all(np.asarray(inputs["b3"]) == 0)
            and np.all(np.asarray(inputs["g1"]) > 0) and np.all(np.asarray(inputs["g2"]) > 0)
            and np.all(np.asarray(inputs["g3"]) > 0))
    if not g_ok:
        raise NotImplementedError("bass path requires beta=0, gamma>0")

    # x splits, padded, x-major: xpad[b, s, 1+xc, 1+yc] = split_s[b, yc, xc]
    xh = x.astype(bf16).astype(np.float32)
    r = x - xh
    xm = r.astype(bf16).astype(np.float32)
    xl = (r - xm).astype(bf16).astype(np.float32)
    xpad = np.zeros((B, 3, 30, 30), np.float32)
    for s, xs in enumerate((xh, xm, xl)):
        xpad[:, s, 1:29, 1:29] = xs.reshape(B, 28, 28).transpose(0, 2, 1)
    xpad = xpad.astype(bf16)

    w1s = sgn(inputs["conv1_w"])[:, 0]          # [32, 3(dy), 3(dx)]
    w1l = np.zeros((3, 7, 90, 128), np.float32)
    for dy in range(3):
        for mt in range(7):
            for xlq in range(4):
                xhat = 4 * mt + xlq
                for dx in range(3):
                    for s in range(3):
                        w1l[dy, mt, s * 30 + xhat + dx,
                            xlq * 32:(xlq + 1) * 32] = w1s[:, dy, dx]
    w1l = w1l.astype(bf16)

    b1 = np.asarray(inputs["conv1_b"], np.float32)
    b1v = np.tile(b1, 4).reshape(128, 1).astype(np.float32)

    w2s = sgn(inputs["conv2_w"])                # [64, 32, 3, 3]
    w2l = np.zeros((9, 128, 128), np.float32)
    for tap in range(9):
        dx, dy = tap // 3, tap % 3
        blk = w2s[:, :, dy, dx]                 # [oc, ic]
        for i in range(4):
            for j in range(4):
                h = j % 2
                w2l[tap, 32 * i:32 * (i + 1),
                    32 * j:32 * (j + 1)] = blk[h * 32:(h + 1) * 32, :].T
    w2l = w2l.astype(fp8)

    # device pixel order is x-major (pix = xw*13 + yw); reference flatten is
    # y-major (feat = c*169 + y*13 + x) -> transpose the spatial dims.
    w3s = sgn(inputs["lin3_w"]).reshape(U, 64, 13, 13).transpose(0, 1, 3, 2)
    w3s = np.ascontiguousarray(w3s).reshape(U, 64, 169)
    w3p = np.zeros((U, 64, 170), np.float32)
    w3p[:, :, :169] = w3s
    # w3i[core][par*64+c, j, mt, m] = w3p[core*256+mt*128+m, c, 2j+par]
    w3pc = w3p.reshape(N_CORES, 2, 128, 64, 85, 2)     # [core, mt, m, c, j, par]
    w3i_all = np.ascontiguousarray(
        w3pc.transpose(0, 5, 3, 4, 1, 2).reshape(N_CORES, 128, 85, 2, 128)
    ).astype(fp8)

    b3 = np.asarray(inputs["lin3_b"], np.float32)
    b3v_all = np.ascontiguousarray(
        (0.25 * b3).reshape(N_CORES, 2, 128).transpose(0, 2, 1)).astype(np.float32)

    w4s = sgn(inputs["lin4_w"])                 # [10, 2048]
    w4i_all = np.ascontiguousarray(w4s.T.reshape(N_CORES, 2, 128, 10)).astype(fp8)

    in_maps = []
    for c in range(N_CORES):
        in_maps.append({
            "xpad": np.ascontiguousarray(xpad[c * BC:(c + 1) * BC]),
            "w1l": w1l, "b1v": b1v, "w2l": w2l,
            "w3i": np.ascontiguousarray(w3i_all[c]),
            "b3v": np.ascontiguousarray(b3v_all[c]),
            "w4i": np.ascontiguousarray(w4i_all[c]),
        })
    return in_maps


def _run_bass(inputs):
    from concourse.bass_utils import run_bass_kernel_spmd
    if "nc" not in _CACHE:
        _CACHE["nc"] = _build_nc()
    in_maps = _host_prep(inputs)
    res = run_bass_kernel_spmd(_CACHE["nc"], in_maps, list(range(N_CORES)))
    parts = np.stack([res.results[c]["out_p"] for c in range(N_CORES)])
    b4 = np.asarray(inputs["lin4_b"], np.float32)
    return (2.0 * parts.sum(axis=0) + b4).astype(np.float32)


# ---------------- jax fallback (previous baseline) ----------------

def _jax_fallback(inputs):
    import jax
    import jax.numpy as jnp
    from functools import partial
    EPS = 1e-4

    def _binarize(x):
        return jnp.where(x >= 0, 1.0, -1.0).astype(x.dtype)

    def _ap2(x):
        a = jnp.maximum(jnp.abs(x), 1e-38)
        return jnp.sign(x) * jnp.exp2(jnp.round(jnp.log2(a)))

    def _shift_bn(x, gamma, beta, axes, axis_name):
        mean = jnp.mean(x, axis=axes, keepdims=True)
        if axis_name is not None:
            mean = jax.lax.pmean(mean, axis_name)
        c = x - mean
        var = jnp.mean(c * _ap2(c), axis=axes, keepdims=True)
        if axis_name is not None:
            var = jax.lax.pmean(var, axis_name)
        xhat = c * _ap2(1.0 / jnp.sqrt(var + EPS))
        shape = [1] * x.ndim
        shape[1] = x.shape[1]
        return _ap2(gamma.reshape(shape)) * xhat + beta.reshape(shape)

    def _conv_bin(x, w, b):
        y = jax.lax.conv_general_dilated(
            x, _binarize(w), (1, 1), [(1, 1), (1, 1)],
            dimension_numbers=("NCHW", "OIHW", "NCHW"))
        return y + b[None, :, None, None]

    def _forward(x, conv1_w, conv1_b, g1, b1, conv2_w, conv2_b, g2, b2,
                 lin3_w, lin3_b, g3, b3, lin4_w, lin4_b, axis_name=None):
        h = x.reshape(-1, 1, 28, 28)
        h = jax.nn.relu(_conv_bin(h, conv1_w, conv1_b))
        h = _shift_bn(h, g1, b1, (0, 2, 3), axis_name)
        h = _binarize(h)
        h = jax.nn.relu(_conv_bin(h, conv2_w, conv2_b))
        h = jax.lax.reduce_window(h, -jnp.inf, jax.lax.max,
                                  (1, 1, 3, 3), (1, 1, 2, 2), "VALID")
        h = _shift_bn(h, g2, b2, (0, 2, 3), axis_name)
        h = _binarize(h)
        h = h.reshape(h.shape[0], -1)
        h = jax.nn.relu(h @ _binarize(lin3_w).T + lin3_b)
        h = _shift_bn(h, g3, b3, (0,), axis_name)
        h = _binarize(h)
        return h @ _binarize(lin4_w).T + lin4_b

    order = ("conv1_w", "conv1_b", "g1", "b1", "conv2_w", "conv2_b", "g2", "b2",
             "lin3_w", "lin3_b", "g3", "b3", "lin4_w", "lin4_b")
    x = np.asarray(inputs["x"], np.float32)
    ws = [np.asarray(inputs[k], np.float32) for k in order]
    if "jax_fn" not in _CACHE:
        devs = [d for d in jax.devices() if d.platform != "cpu"][:N_CORES]
        if len(devs) < N_CORES:
            devs = jax.devices()[:N_CORES]
        _CACHE["jax_fn"] = jax.pmap(
            partial(_forward, axis_name="i"), axis_name="i",
            in_axes=(0,) + (None,) * len(order), devices=devs)
    out = _CACHE["jax_fn"](x.reshape(N_CORES, BC, 784), *ws)
    return np.asarray(out, np.float32).reshape(B, 10)


def kernel(**inputs):
    try:
        return _run_bass(inputs)
    except Exception:
        import traceback
        traceback.print_exc()
        return _jax_fallback(inputs)
